# revision 6
# baseline (speedup 1.0000x reference)
"""GNN edge-scorer (MLPPredictor) Trainium2 kernel, v4.

score[e, :] = h[src[e]] @ Wu.T + h[dst[e]] @ Wv.T + b

Strategy: per-side partial scores over UNIQUE nodes with sorted dma_gather.
  - h cast to bf16 [100000, 128]; W split/transposed to wut/wvt bf16 [128, 64].
  - Per core (75000 edges), per side (u=src, v=dst):
      * Host dedups the side's nodes (~53k unique of 75k draws; edges sharing
        a node share one slot) and lays the sorted unique indices out in 4
        buckets of 32768 rows (int16-addressable via per-bucket base), each
        padded to a fixed cap -> static SPMD layout of T=54784 slots = 428
        subtiles = 13 blocks of 4096 + 1 of 1536. Sorted unique indices also
        make each gather's HBM reads monotone (no duplicate-row hammering).
      * Per block: 1-2 dma_gather(transpose=True) ops (static base/num_idxs)
        pull h rows for 4096 slots directly into [d, slots] bf16 layout.
      * Per subtile: matmul (lhsT = gathered [128d, 128slots], rhs = w*t)
        into a shared f32 PSUM bank (8 subtiles per bank); bias row added via
        K=1 ones-matmul on the u side; one ACT (u) / DVE (v) copy per bank
        into a bf16 score slab; one store per block.
  - Host: fan per-node partials back out to edges via the slot map and add
    u+v in f32.
"""

import numpy as np
import ml_dtypes

import concourse.bacc as bacc
import concourse.bass as bass
import concourse.mybir as mybir
import concourse.tile as tile
from concourse.bass_utils import run_bass_kernel_spmd

N_CORES = 8
N_NODES = 100000
N_EDGES = 600000
D = 128
C = 64
EDGES_PER_CORE = N_EDGES // N_CORES   # 75000

BUCKET_BITS = 15
BUCKET = 1 << BUCKET_BITS             # 32768
BUCKET_SIZES = [BUCKET, BUCKET, BUCKET, N_NODES - 3 * BUCKET]  # last: 1696
CAPS = [17920, 17920, 17920, 1024]    # per-bucket UNIQUE-node caps (x128)
CAP_STARTS = [0, 17920, 35840, 53760]
T_SLOTS = sum(CAPS)                   # 54784
N_SUB = T_SLOTS // 128                # 428
S_MAIN = 32
N_MAIN = 13                           # 13 blocks of 4096 slots
S_TAIL = 12                           # + 1 block of 1536 slots
BLOCK_SLOTS = S_MAIN * 128

_F32 = mybir.dt.float32
_BF16 = mybir.dt.bfloat16
_I16 = mybir.dt.int16

_CACHE: dict = {}


def _block_runs():
    """Static (slot0, S, [(bucket, base_slot_in_bucket, ni, dstoff), ...])."""
    bounds = CAP_STARTS + [T_SLOTS]
    blocks = []
    for t in range(N_MAIN + 1):
        slot0 = t * BLOCK_SLOTS
        S = S_MAIN if t < N_MAIN else S_TAIL
        end = slot0 + S * 128
        runs = []
        for k in range(4):
            lo = max(slot0, bounds[k])
            hi = min(end, bounds[k + 1])
            if lo < hi:
                runs.append((k, lo, hi - lo, lo - slot0))
        blocks.append((slot0, S, runs))
    return blocks


BLOCKS = _block_runs()


def build_nc(reps: int = 1, probe: int = 0):
    # probe=1: gathers+stores only; probe=2: tiny gathers + full compute
    if ("nc", reps, probe) in _CACHE:
        return _CACHE[("nc", reps, probe)]
    nc = bacc.Bacc("TRN2", target_bir_lowering=False)
    h = nc.dram_tensor("h", [N_NODES, D], _BF16, kind="ExternalInput")
    idx_u = nc.dram_tensor("idx_u", [128, T_SLOTS // 16], _I16, kind="ExternalInput")
    idx_v = nc.dram_tensor("idx_v", [128, T_SLOTS // 16], _I16, kind="ExternalInput")
    wut = nc.dram_tensor("wut", [D, C], _BF16, kind="ExternalInput")
    wvt = nc.dram_tensor("wvt", [D, C], _BF16, kind="ExternalInput")
    brow = nc.dram_tensor("brow", [1, C], _BF16, kind="ExternalInput")
    outs = {}
    for side in ("u", "v"):
        outs[side] = (
            nc.dram_tensor(f"out_{side}_main", [N_MAIN, 128, S_MAIN * C], _BF16,
                           kind="ExternalOutput"),
            nc.dram_tensor(f"out_{side}_tail", [128, S_TAIL * C], _BF16,
                           kind="ExternalOutput"),
        )

    with tile.TileContext(nc) as tc:
        with (
            tc.tile_pool(name="const", bufs=1) as cpool,
            tc.tile_pool(name="gather", bufs=4) as gpool,
            tc.tile_pool(name="score", bufs=3) as spool,
            tc.tile_pool(name="psum_s", bufs=4, space="PSUM") as pspool,
        ):
            wut_sb = cpool.tile([D, C], _BF16, tag="wut")
            nc.sync.dma_start(out=wut_sb[:], in_=wut[:, :])
            wvt_sb = cpool.tile([D, C], _BF16, tag="wvt")
            nc.sync.dma_start(out=wvt_sb[:], in_=wvt[:, :])
            brow_sb = cpool.tile([1, C], _BF16, tag="brow")
            nc.sync.dma_start(out=brow_sb[:], in_=brow[:, :])
            ones_sb = cpool.tile([1, 128], _BF16, tag="ones")
            nc.vector.memset(ones_sb[:], 1.0)
            idx_sb = {}
            for side, dram in (("u", idx_u), ("v", idx_v)):
                t_ = cpool.tile([128, T_SLOTS // 16], _I16, tag=f"idx_{side}")
                nc.sync.dma_start(out=t_[:], in_=dram[:, :])
                idx_sb[side] = t_

            g_i = 0
            for rep in range(reps):
              for side in ("u", "v"):
                w_sb = wut_sb if side == "u" else wvt_sb
                out_main, out_tail = outs[side]
                for t, (slot0, S, runs) in enumerate(BLOCKS):
                    gT = gpool.tile([128, 1, BLOCK_SLOTS], _BF16, tag="g")
                    if probe == 2:
                        runs = runs[:1]
                    for k, lo, ni, dstoff in runs:
                        if probe == 2:
                            ni, dstoff = 128, 0
                        nc.gpsimd.dma_gather(
                            out_ap=gT[:, :, dstoff:dstoff + ni],
                            in_ap=h[k * BUCKET:k * BUCKET + BUCKET_SIZES[k], :],
                            idxs_ap=idx_sb[side][:, lo // 16:(lo + ni) // 16],
                            num_idxs=ni,
                            num_idxs_reg=ni,
                            elem_size=D,
                            transpose=True,
                            single_packet=False,
                        )
                    score = spool.tile([128, S_MAIN * C], _BF16, tag="sc")
                    if probe == 1:
                        nc.vector.memset(score[:, :1], 0.0)
                    for si in (range(0, S, 8) if probe != 1 else []):
                        gs8 = min(8, S - si)
                        psb = pspool.tile([128, 8 * C], _F32, tag="ps")
                        for j in range(gs8):
                            s = si + j
                            ps = psb[:, j * C:(j + 1) * C]
                            lhsT = gT[:, 0, s * 128:(s + 1) * 128]
                            if side == "u":
                                nc.tensor.matmul(
                                    ps, lhsT=ones_sb[:], rhs=brow_sb[:],
                                    start=True, stop=False,
                                )
                                nc.tensor.matmul(
                                    ps, lhsT=lhsT, rhs=w_sb[:],
                                    start=False, stop=True,
                                )
                            else:
                                nc.tensor.matmul(
                                    ps, lhsT=lhsT, rhs=w_sb[:],
                                    start=True, stop=True,
                                )
                        if side == "u":
                            nc.scalar.copy(
                                out=score[:, si * C:(si + gs8) * C],
                                in_=psb[:, : gs8 * C],
                            )
                        else:
                            nc.vector.tensor_copy(
                                score[:, si * C:(si + gs8) * C],
                                psb[:, : gs8 * C],
                            )
                    if S == S_MAIN:
                        nc.sync.dma_start(out=out_main[t, :, :], in_=score[:])
                    else:
                        nc.sync.dma_start(
                            out=out_tail[:, :], in_=score[:, : S * C]
                        )

    nc.finalize()
    _CACHE[("nc", reps, probe)] = nc
    return nc


def _side_prep(nodes: np.ndarray):
    """One slot per UNIQUE node (edges sharing a node share a slot; the
    host-side slot map fans the computed partial back out to every edge)."""
    nodes = nodes.astype(np.int64)
    uniq, inv = np.unique(nodes, return_inverse=True)  # uniq sorted ascending
    bucket_u = uniq >> BUCKET_BITS
    counts = np.bincount(bucket_u, minlength=4)
    assert all(counts[k] <= CAPS[k] for k in range(4)), counts
    rel_u = (uniq - (bucket_u << BUCKET_BITS)).astype(np.int16)

    idx_flat = np.zeros(T_SLOTS, dtype=np.int16)
    slot_of_unique = np.empty(len(uniq), dtype=np.int64)
    pos = 0
    for k in range(4):
        idx_flat[CAP_STARTS[k]:CAP_STARTS[k] + counts[k]] = rel_u[pos:pos + counts[k]]
        slot_of_unique[pos:pos + counts[k]] = CAP_STARTS[k] + np.arange(counts[k])
        pos += counts[k]
    slot_of_edge = slot_of_unique[inv]

    wrapped = np.tile(idx_flat.reshape(T_SLOTS // 16, 16).T, (8, 1))
    return np.ascontiguousarray(wrapped), slot_of_edge


def make_in_maps(h, src, dst, W, b):
    h_bf = np.ascontiguousarray(np.asarray(h, dtype=np.float32)).astype(
        ml_dtypes.bfloat16
    )
    W = np.asarray(W, dtype=np.float32)
    b = np.asarray(b, dtype=np.float32)
    wut = np.ascontiguousarray(W[:, :D].T).astype(ml_dtypes.bfloat16)
    wvt = np.ascontiguousarray(W[:, D:].T).astype(ml_dtypes.bfloat16)
    brow = b.reshape(1, C).astype(ml_dtypes.bfloat16)

    src = np.asarray(src).astype(np.int64)
    dst = np.asarray(dst).astype(np.int64)

    in_maps = []
    slot_maps = []
    for core in range(N_CORES):
        lo = core * EDGES_PER_CORE
        iu, su = _side_prep(src[lo:lo + EDGES_PER_CORE])
        iv, sv = _side_prep(dst[lo:lo + EDGES_PER_CORE])
        in_maps.append(
            {"h": h_bf, "idx_u": iu, "idx_v": iv, "wut": wut, "wvt": wvt,
             "brow": brow}
        )
        slot_maps.append((su, sv))
    _CACHE["slot_maps"] = slot_maps
    return in_maps


def _side_scores(r, side):
    main = np.asarray(r[f"out_{side}_main"]).reshape(N_MAIN, 128, S_MAIN, C)
    main = main.transpose(0, 2, 1, 3).reshape(N_MAIN * BLOCK_SLOTS, C)
    tail = np.asarray(r[f"out_{side}_tail"]).reshape(128, S_TAIL, C)
    tail = tail.transpose(1, 0, 2).reshape(S_TAIL * 128, C)
    return np.concatenate([main, tail], axis=0)


def assemble_output(results) -> np.ndarray:
    slot_maps = _CACHE["slot_maps"]
    per_core = []
    for core, r in enumerate(results):
        su_map, sv_map = slot_maps[core]
        su = _side_scores(r, "u")[su_map].astype(np.float32)
        sv = _side_scores(r, "v")[sv_map].astype(np.float32)
        per_core.append(su + sv)
    return np.concatenate(per_core, axis=0)


def run(h, src, dst, W, b, **spmd_kwargs):
    nc = build_nc()
    in_maps = make_in_maps(h, src, dst, W, b)
    res = run_bass_kernel_spmd(nc, in_maps, core_ids=list(range(N_CORES)), **spmd_kwargs)
    return assemble_output(res.results), res


def kernel(h, src, dst, W, b):
    out, _ = run(h, src, dst, W, b)
    return out


# revision 7
# speedup vs baseline: 1.0452x; 1.0452x over previous
"""GNN edge-scorer (MLPPredictor) Trainium2 kernel, v4.

score[e, :] = h[src[e]] @ Wu.T + h[dst[e]] @ Wv.T + b

Strategy: per-side partial scores over UNIQUE nodes with sorted dma_gather.
  - h cast to bf16 [100000, 128]; W split/transposed to wut/wvt bf16 [128, 64].
  - Per core (75000 edges), per side (u=src, v=dst):
      * Host dedups the side's nodes (~53k unique of 75k draws; edges sharing
        a node share one slot) and lays the sorted unique indices out in 4
        buckets of 32768 rows (int16-addressable via per-bucket base), each
        padded to a fixed cap -> static SPMD layout of T=54784 slots = 428
        subtiles = 13 blocks of 4096 + 1 of 1536. Sorted unique indices also
        make each gather's HBM reads monotone (no duplicate-row hammering).
      * Per block: 1-2 dma_gather(transpose=True) ops (static base/num_idxs)
        pull h rows for 4096 slots directly into [d, slots] bf16 layout.
      * Per subtile: matmul (lhsT = gathered [128d, 128slots], rhs = w*t)
        into a shared f32 PSUM bank (8 subtiles per bank); bias row added via
        K=1 ones-matmul on the u side; one ACT (u) / DVE (v) copy per bank
        into a bf16 score slab; one store per block.
  - Host: fan per-node partials back out to edges via the slot map and add
    u+v in f32.

HW facts this design is built on (all measured on the axon trn2 cores):
  - indirect_dma_start honors ONE offset per partition; extra offset columns
    are ignored and S*D consecutive elements stream from offset[p,0].
  - dma_gather(single_packet=True) wedges the exec unit beyond ~512 idxs;
    single_packet=False is mandatory.
  - Multi-queue SWDGE (num_swdge_queues>1, queue_num>0) corrupts ALL queues'
    gathers in this runtime (rings for queues 1-3 not provisioned). ~4x
    latent upside if that ever gets fixed.
  - Gather cost is per-index (Q7 descriptor generation), ~5-6 ns/row at 8
    cores after dedup+sort; duplicate rows in an index stream roughly
    triple the per-row cost (hence unique-node slots, not per-edge slots).
  - Verified next step (not yet integrated): elem_size=256 pair-gathers
    from a [50000, 256] bf16 table view run at ~3.1 ns/row; ~53% of deduped
    sorted nodes have their even-aligned partner present -> expected ~15-20%
    end-to-end win via a paired+single two-segment slot layout.
  - Per-bucket caps (17920/1024) are statistical (5 sigma over the fixed
    seed-0 graph: max observed 17487/964); pathological all-unique graphs
    would need caps at 32768 and ~30% more slots.
"""

import numpy as np
import ml_dtypes

import concourse.bacc as bacc
import concourse.bass as bass
import concourse.mybir as mybir
import concourse.tile as tile
from concourse.bass_utils import run_bass_kernel_spmd

N_CORES = 8
N_NODES = 100000
N_EDGES = 600000
D = 128
C = 64
EDGES_PER_CORE = N_EDGES // N_CORES   # 75000

BUCKET_BITS = 15
BUCKET = 1 << BUCKET_BITS             # 32768
BUCKET_SIZES = [BUCKET, BUCKET, BUCKET, N_NODES - 3 * BUCKET]  # last: 1696
CAPS = [17920, 17920, 17920, 1024]    # per-bucket UNIQUE-node caps (x128)
CAP_STARTS = [0, 17920, 35840, 53760]
T_SLOTS = sum(CAPS)                   # 54784
N_SUB = T_SLOTS // 128                # 428
S_MAIN = 32
N_MAIN = 13                           # 13 blocks of 4096 slots
S_TAIL = 12                           # + 1 block of 1536 slots
BLOCK_SLOTS = S_MAIN * 128

_F32 = mybir.dt.float32
_BF16 = mybir.dt.bfloat16
_I16 = mybir.dt.int16

_CACHE: dict = {}


def _block_runs():
    """Static (slot0, S, [(bucket, base_slot_in_bucket, ni, dstoff), ...])."""
    bounds = CAP_STARTS + [T_SLOTS]
    blocks = []
    for t in range(N_MAIN + 1):
        slot0 = t * BLOCK_SLOTS
        S = S_MAIN if t < N_MAIN else S_TAIL
        end = slot0 + S * 128
        runs = []
        for k in range(4):
            lo = max(slot0, bounds[k])
            hi = min(end, bounds[k + 1])
            if lo < hi:
                runs.append((k, lo, hi - lo, lo - slot0))
        blocks.append((slot0, S, runs))
    return blocks


BLOCKS = _block_runs()


def build_nc(reps: int = 1, probe: int = 0):
    # probe=1: gathers+stores only; probe=2: tiny gathers + full compute
    if ("nc", reps, probe) in _CACHE:
        return _CACHE[("nc", reps, probe)]
    nc = bacc.Bacc("TRN2", target_bir_lowering=False)
    h = nc.dram_tensor("h", [N_NODES, D], _BF16, kind="ExternalInput")
    idx_u = nc.dram_tensor("idx_u", [128, T_SLOTS // 16], _I16, kind="ExternalInput")
    idx_v = nc.dram_tensor("idx_v", [128, T_SLOTS // 16], _I16, kind="ExternalInput")
    wut = nc.dram_tensor("wut", [D, C], _BF16, kind="ExternalInput")
    wvt = nc.dram_tensor("wvt", [D, C], _BF16, kind="ExternalInput")
    brow = nc.dram_tensor("brow", [1, C], _BF16, kind="ExternalInput")
    outs = {}
    for side in ("u", "v"):
        outs[side] = (
            nc.dram_tensor(f"out_{side}_main", [N_MAIN, 128, S_MAIN * C], _BF16,
                           kind="ExternalOutput"),
            nc.dram_tensor(f"out_{side}_tail", [128, S_TAIL * C], _BF16,
                           kind="ExternalOutput"),
        )

    with tile.TileContext(nc) as tc:
        with (
            tc.tile_pool(name="const", bufs=1) as cpool,
            tc.tile_pool(name="gather", bufs=4) as gpool,
            tc.tile_pool(name="score", bufs=3) as spool,
            tc.tile_pool(name="psum_s", bufs=4, space="PSUM") as pspool,
        ):
            wut_sb = cpool.tile([D, C], _BF16, tag="wut")
            nc.sync.dma_start(out=wut_sb[:], in_=wut[:, :])
            wvt_sb = cpool.tile([D, C], _BF16, tag="wvt")
            nc.sync.dma_start(out=wvt_sb[:], in_=wvt[:, :])
            brow_sb = cpool.tile([1, C], _BF16, tag="brow")
            nc.sync.dma_start(out=brow_sb[:], in_=brow[:, :])
            ones_sb = cpool.tile([1, 128], _BF16, tag="ones")
            nc.vector.memset(ones_sb[:], 1.0)
            idx_sb = {}
            for side, dram in (("u", idx_u), ("v", idx_v)):
                t_ = cpool.tile([128, T_SLOTS // 16], _I16, tag=f"idx_{side}")
                nc.sync.dma_start(out=t_[:], in_=dram[:, :])
                idx_sb[side] = t_

            g_i = 0
            for rep in range(reps):
              for side in ("u", "v"):
                w_sb = wut_sb if side == "u" else wvt_sb
                out_main, out_tail = outs[side]
                for t, (slot0, S, runs) in enumerate(BLOCKS):
                    gT = gpool.tile([128, 1, BLOCK_SLOTS], _BF16, tag="g")
                    if probe == 2:
                        runs = runs[:1]
                    for k, lo, ni, dstoff in runs:
                        if probe == 2:
                            ni, dstoff = 128, 0
                        nc.gpsimd.dma_gather(
                            out_ap=gT[:, :, dstoff:dstoff + ni],
                            in_ap=h[k * BUCKET:k * BUCKET + BUCKET_SIZES[k], :],
                            idxs_ap=idx_sb[side][:, lo // 16:(lo + ni) // 16],
                            num_idxs=ni,
                            num_idxs_reg=ni,
                            elem_size=D,
                            transpose=True,
                            single_packet=False,
                        )
                    score = spool.tile([128, S_MAIN * C], _BF16, tag="sc")
                    if probe == 1:
                        nc.vector.memset(score[:, :1], 0.0)
                    for si in (range(0, S, 8) if probe != 1 else []):
                        gs8 = min(8, S - si)
                        psb = pspool.tile([128, 8 * C], _F32, tag="ps")
                        for j in range(gs8):
                            s = si + j
                            ps = psb[:, j * C:(j + 1) * C]
                            lhsT = gT[:, 0, s * 128:(s + 1) * 128]
                            if side == "u":
                                nc.tensor.matmul(
                                    ps, lhsT=ones_sb[:], rhs=brow_sb[:],
                                    start=True, stop=False,
                                )
                                nc.tensor.matmul(
                                    ps, lhsT=lhsT, rhs=w_sb[:],
                                    start=False, stop=True,
                                )
                            else:
                                nc.tensor.matmul(
                                    ps, lhsT=lhsT, rhs=w_sb[:],
                                    start=True, stop=True,
                                )
                        if side == "u":
                            nc.scalar.copy(
                                out=score[:, si * C:(si + gs8) * C],
                                in_=psb[:, : gs8 * C],
                            )
                        else:
                            nc.vector.tensor_copy(
                                score[:, si * C:(si + gs8) * C],
                                psb[:, : gs8 * C],
                            )
                    if S == S_MAIN:
                        nc.sync.dma_start(out=out_main[t, :, :], in_=score[:])
                    else:
                        nc.sync.dma_start(
                            out=out_tail[:, :], in_=score[:, : S * C]
                        )

    nc.finalize()
    _CACHE[("nc", reps, probe)] = nc
    return nc


def _side_prep(nodes: np.ndarray):
    """One slot per UNIQUE node (edges sharing a node share a slot; the
    host-side slot map fans the computed partial back out to every edge)."""
    nodes = nodes.astype(np.int64)
    uniq, inv = np.unique(nodes, return_inverse=True)  # uniq sorted ascending
    bucket_u = uniq >> BUCKET_BITS
    counts = np.bincount(bucket_u, minlength=4)
    assert all(counts[k] <= CAPS[k] for k in range(4)), counts
    rel_u = (uniq - (bucket_u << BUCKET_BITS)).astype(np.int16)

    idx_flat = np.zeros(T_SLOTS, dtype=np.int16)
    slot_of_unique = np.empty(len(uniq), dtype=np.int64)
    pos = 0
    for k in range(4):
        idx_flat[CAP_STARTS[k]:CAP_STARTS[k] + counts[k]] = rel_u[pos:pos + counts[k]]
        slot_of_unique[pos:pos + counts[k]] = CAP_STARTS[k] + np.arange(counts[k])
        pos += counts[k]
    slot_of_edge = slot_of_unique[inv]

    wrapped = np.tile(idx_flat.reshape(T_SLOTS // 16, 16).T, (8, 1))
    return np.ascontiguousarray(wrapped), slot_of_edge


def make_in_maps(h, src, dst, W, b):
    h_bf = np.ascontiguousarray(np.asarray(h, dtype=np.float32)).astype(
        ml_dtypes.bfloat16
    )
    W = np.asarray(W, dtype=np.float32)
    b = np.asarray(b, dtype=np.float32)
    wut = np.ascontiguousarray(W[:, :D].T).astype(ml_dtypes.bfloat16)
    wvt = np.ascontiguousarray(W[:, D:].T).astype(ml_dtypes.bfloat16)
    brow = b.reshape(1, C).astype(ml_dtypes.bfloat16)

    src = np.asarray(src).astype(np.int64)
    dst = np.asarray(dst).astype(np.int64)

    in_maps = []
    slot_maps = []
    for core in range(N_CORES):
        lo = core * EDGES_PER_CORE
        iu, su = _side_prep(src[lo:lo + EDGES_PER_CORE])
        iv, sv = _side_prep(dst[lo:lo + EDGES_PER_CORE])
        in_maps.append(
            {"h": h_bf, "idx_u": iu, "idx_v": iv, "wut": wut, "wvt": wvt,
             "brow": brow}
        )
        slot_maps.append((su, sv))
    _CACHE["slot_maps"] = slot_maps
    return in_maps


def _side_scores(r, side):
    main = np.asarray(r[f"out_{side}_main"]).reshape(N_MAIN, 128, S_MAIN, C)
    main = main.transpose(0, 2, 1, 3).reshape(N_MAIN * BLOCK_SLOTS, C)
    tail = np.asarray(r[f"out_{side}_tail"]).reshape(128, S_TAIL, C)
    tail = tail.transpose(1, 0, 2).reshape(S_TAIL * 128, C)
    return np.concatenate([main, tail], axis=0)


def assemble_output(results) -> np.ndarray:
    slot_maps = _CACHE["slot_maps"]
    per_core = []
    for core, r in enumerate(results):
        su_map, sv_map = slot_maps[core]
        su = _side_scores(r, "u")[su_map].astype(np.float32)
        sv = _side_scores(r, "v")[sv_map].astype(np.float32)
        per_core.append(su + sv)
    return np.concatenate(per_core, axis=0)


def run(h, src, dst, W, b, **spmd_kwargs):
    nc = build_nc()
    in_maps = make_in_maps(h, src, dst, W, b)
    res = run_bass_kernel_spmd(nc, in_maps, core_ids=list(range(N_CORES)), **spmd_kwargs)
    return assemble_output(res.results), res


def kernel(h, src, dst, W, b):
    out, _ = run(h, src, dst, W, b)
    return out
